# revision 47
# baseline (speedup 1.0000x reference)
"""Neural CDE Trainium2 kernel.

Data-parallel over batch: 8 cores x 64 batch. Per core, two independent
"chains" of 32 batch elements run the 255-step Euler scan concurrently so
engine work from one chain hides the serial-dependency latency of the other.

Critical-cycle-oriented design (wall time ~= per-chain dependency cycle,
not engine throughput; fp32 everywhere - the CDE dynamics are chaotic
(error amplification ~1e8 over 255 steps) so bf16/fp32r anywhere explodes
the output):
  state s[f, b] = W1^T z + b1, updated incrementally via the linearity of
  MM1:  s_{k+1} = s_k + W1^T zinc_k.  The z state itself is accumulated
  OFF the critical path on Pool; only s is on the cycle.

Per-chain layouts (b in [0,32), h = 32*g + h_lo, g in [0,4)):
  ds = W1^T zit [128, 32]  PSUM, one K=128 matmul (stationary W1 [h, f])
  s  = s + ds   [128, 32]  SBUF (DVE; k=0 adds b1 per-partition instead)
  h = max(s, 0) [128, 32]  SBUF (DVE tensor_scalar)
  y = b2 + h@W2 [128, 384] PSUM, partitions p = 32*g + b, free = (h_lo, c);
                one K=4 bias-seed matmul (rank-4 factorization b2big =
                E4^T B4, start=True, no data deps so it runs early) + per
                group g a K=128 main matmul (stop=True); the 4 groups
                stream CONCURRENTLY via col tiling at tile_position
                (0, 32g) (HW-verified ~4x vs serial)
  t = tanh(y)   [128, 384] SBUF (ACT reads PSUM)
  u = t * dx    [128, 384] SBUF (chain 0: DVE, chain 1: Pool — splits the
                two chains' long tails across engine queues; dx broadcast
                over h_lo via 0-stride AP)
  zi            [128, 32]  SBUF (DVE reduce over innermost c) = [(g,b), h_lo]
  zit           [128, 32]  SBUF (DVE 32x32 block transpose) = [h, b] = zinc
  z_new = z + zit          (Pool, off critical path; z only needed at end)

z0, dX and the final readout/softmax are tiny (<0.01% of FLOPs) and run on
host numpy as part of sharding/gather.
"""

import numpy as np
from contextlib import ExitStack

B, T, C = 512, 256, 12
H, FF, O = 128, 128, 20
NCORES = 8
BLOC = B // NCORES          # 64 batch per core
NCHAIN = 2
BCH = BLOC // NCHAIN        # 32 batch per chain
NSTEP = T - 1               # 255
NG = 4                      # h groups of 32
FD = BCH * C                # 384 free elems for y/t/u tiles

_CACHE = {}


def _build(nsteps=NSTEP):
    import concourse.bass as bass
    import concourse.mybir as mybir
    import concourse.tile as tile
    import concourse.bacc as bacc

    f32 = mybir.dt.float32

    nc = bacc.Bacc("TRN2", target_bir_lowering=False, debug=False,
                   num_devices=NCORES)

    z0T_d = nc.dram_tensor("z0T", [NCHAIN, H, BCH], f32, kind="ExternalInput")
    W1_d = nc.dram_tensor("W1", [H, FF], f32, kind="ExternalInput")
    b1_d = nc.dram_tensor("b1", [FF, 1], f32, kind="ExternalInput")
    W2_d = nc.dram_tensor("W2", [FF, H * C], f32, kind="ExternalInput")
    # rank-4 factorization of the per-partition-group bias: y init = E4^T B4
    e4_d = nc.dram_tensor("e4", [NG, 128], f32, kind="ExternalInput")
    b4_d = nc.dram_tensor("b4", [NG, FD], f32, kind="ExternalInput")
    nd = NSTEP
    dx_d = nc.dram_tensor("dxs", [NCHAIN, 128, nd * C], f32,
                          kind="ExternalInput")
    zout_d = nc.dram_tensor("zT_out", [NCHAIN, H, BCH], f32,
                            kind="ExternalOutput")

    with tile.TileContext(nc) as tc, ExitStack() as ctx:
        consts = ctx.enter_context(tc.tile_pool(name="consts", bufs=1))

        w1 = consts.tile([H, FF], f32, tag="w1")
        nc.sync.dma_start(w1[:], W1_d[:])
        b1c = consts.tile([FF, 1], f32, tag="b1c")
        nc.sync.dma_start(b1c[:], b1_d[:])
        w2 = consts.tile([FF, H * C], f32, tag="w2")
        nc.sync.dma_start(w2[:], W2_d[:])
        e4 = consts.tile([NG, 128], f32, tag="e4")
        nc.sync.dma_start(e4[:], e4_d[:])
        b4 = consts.tile([NG, FD], f32, tag="b4")
        nc.sync.dma_start(b4[:], b4_d[:])
        dxs = [consts.tile([128, nd * C], f32, tag=f"dx{q}", name=f"dx{q}")
               for q in range(NCHAIN)]
        for q in range(NCHAIN):
            nc.sync.dma_start(dxs[q][:], dx_d[q])

        pools = {}
        for q in range(NCHAIN):
            pools[q] = {
                "zt": ctx.enter_context(tc.tile_pool(name=f"zt{q}", bufs=2)),
                "h": ctx.enter_context(tc.tile_pool(name=f"h{q}", bufs=2)),
                "t": ctx.enter_context(tc.tile_pool(name=f"t{q}", bufs=2)),
                "u": ctx.enter_context(tc.tile_pool(name=f"u{q}", bufs=2)),
                "zi": ctx.enter_context(tc.tile_pool(name=f"zi{q}", bufs=2)),
                "zit": ctx.enter_context(tc.tile_pool(name=f"zit{q}", bufs=2)),
                "s": ctx.enter_context(tc.tile_pool(name=f"ssb{q}", bufs=2)),
                "ds": ctx.enter_context(tc.tile_pool(
                    name=f"dsps{q}", bufs=2, space=bass.MemorySpace.PSUM)),
                "y": ctx.enter_context(tc.tile_pool(
                    name=f"yps{q}", bufs=2, space=bass.MemorySpace.PSUM)),
            }

        # initial z state; s = W1^T z + b1 lives in SBUF, updated via the
        # MM1 linearity trick each step
        zt, s_st = [], [None] * NCHAIN
        for q in range(NCHAIN):
            z = pools[q]["zt"].tile([H, BCH], f32, tag="zt", name=f"zt_init{q}")
            nc.sync.dma_start(z[:], z0T_d[q])
            zt.append(z)

        zit_prev = [None] * NCHAIN
        u_st = [None] * NCHAIN

        def seed(q):
            """y-tile bias seed via one K=4 matmul. Emitted for BOTH chains
            before either chain's stall-prone MM1-delta so the in-order PE
            fills its zit-wait window with this work."""
            p = pools[q]
            y = p["y"].tile([128, FD], f32, tag="y", name=f"y{q}")
            nc.tensor.matmul(y[:], e4[:], b4[:],
                             start=True, stop=False, skip_group_check=True)
            return y

        def head(q, k, y):
            """MM1 delta + s update + relu + MM2 (PE/DVE)."""
            p = pools[q]
            src = zt[q] if k == 0 else zit_prev[q]
            ds = p["ds"].tile([FF, BCH], f32, tag="ds")
            nc.tensor.matmul(ds[:], w1[:], src[:], start=True, stop=True)
            snew = p["s"].tile([FF, BCH], f32, tag="s")
            if k == 0:
                nc.vector.tensor_scalar(snew[:], ds[:], b1c[:], None,
                                        mybir.AluOpType.add)
            else:
                nc.vector.tensor_add(snew[:], s_st[q][:], ds[:])
            s_st[q] = snew
            h = p["h"].tile([FF, BCH], f32, tag="h")
            nc.vector.tensor_scalar(h[:], snew[:], 0.0, None,
                                    mybir.AluOpType.max)
            for g in range(NG):
                nc.tensor.matmul(y[32 * g:32 * (g + 1), :], h[:],
                                 w2[:, FD * g:FD * (g + 1)],
                                 start=False, stop=True,
                                 tile_position=(0, 32 * g),
                                 skip_group_check=True)
            return y

        def tanh_mul(q, k, y):
            """tanh (ACT) + dx multiply (chain 0: DVE, chain 1: Pool)."""
            p = pools[q]
            t = p["t"].tile([128, FD], f32, tag="t")
            nc.scalar.activation(t[:], y[:],
                                 mybir.ActivationFunctionType.Tanh)
            veng = nc.vector if q == 0 else nc.gpsimd
            u = p["u"].tile([128, FD], f32, tag="u")
            kk = k % nd
            dxk = dxs[q][:, C * kk:C * (kk + 1)]
            veng.tensor_tensor(
                u[:].rearrange("p (hl c) -> p hl c", c=C),
                t[:].rearrange("p (hl c) -> p hl c", c=C),
                dxk.unsqueeze(1).broadcast_to([128, BCH, C]),
                mybir.AluOpType.mult)
            u_st[q] = u

        def red_xp(q):
            """c-reduce + 32x32 block transpose (DVE) + z accumulate (Pool)."""
            p = pools[q]
            zi = p["zi"].tile([128, BCH], f32, tag="zi")
            nc.vector.reduce_sum(zi[:],
                                 u_st[q][:].rearrange("p (hl c) -> p hl c", c=C),
                                 axis=mybir.AxisListType.X)
            zit = p["zit"].tile([128, BCH], f32, tag="zit")
            nc.vector.transpose(zit[:], zi[:])
            zit_prev[q] = zit
            znew = p["zt"].tile([H, BCH], f32, tag="zt")
            nc.gpsimd.tensor_add(znew[:], zt[q][:], zit[:])
            zt[q] = znew

        for k in range(nsteps):
            # both chains' seeds, then both heads, then both tails: chain 1's
            # head ops no longer queue behind chain 0's tail in the in-order
            # DVE queue (and nothing is deferred across iterations)
            ys = [seed(q) for q in range(NCHAIN)]
            for q in range(NCHAIN):
                head(q, k, ys[q])
            for q in range(NCHAIN):
                tanh_mul(q, k, ys[q])
                red_xp(q)

        for q in range(NCHAIN):
            nc.sync.dma_start(zout_d[q], zt[q][:])

    nc.compile()
    return nc


def _get_nc(nsteps=NSTEP):
    key = ("nc", nsteps)
    if key not in _CACHE:
        _CACHE[key] = _build(nsteps)
    return _CACHE[key]


def _prep_core(z0, dX, r):
    """Per-core input map. z0 [B, H] fp32, dX [B, T-1, C] fp32."""
    z0c = z0[BLOC * r:BLOC * (r + 1)]          # [64, 128]
    # [q, b, h] -> [q, h, b]
    z0T = (z0c.reshape(NCHAIN, BCH, H)
              .transpose(0, 2, 1)
              .astype(np.float32, copy=True))
    dxc = dX[BLOC * r:BLOC * (r + 1)]          # [64, 255, 12]
    nd = NSTEP
    dxq = np.empty((NCHAIN, 128, nd * C), np.float32)
    for q in range(NCHAIN):
        blk = dxc[BCH * q:BCH * (q + 1), :nd]      # [32, nd, 12]
        dxq[q] = np.tile(blk.reshape(BCH, nd * C), (NG, 1))
    return {"z0T": np.ascontiguousarray(z0T), "dxs": np.ascontiguousarray(dxq)}


def _shared_maps(W1, b1, W2, b2):
    """Per-core replicated weight tensors."""
    b2m = np.asarray(b2, np.float32).reshape(NG, BCH * C)    # chunk g = rows
    e4 = np.kron(np.eye(NG, dtype=np.float32), np.ones((1, 32), np.float32))
    return {
        "W1": np.ascontiguousarray(np.asarray(W1, np.float32)),
        "b1": np.ascontiguousarray(np.asarray(b1, np.float32).reshape(FF, 1)),
        "W2": np.ascontiguousarray(np.asarray(W2, np.float32)),
        "e4": np.ascontiguousarray(e4.reshape(NG, 128)),
        "b4": np.ascontiguousarray(b2m),
    }


def kernel(coeffs, times, W_init, b_init, W1, b1, W2, b2, W_out, b_out,
           _want_results=False):
    from concourse.bass_utils import run_bass_kernel_spmd

    coeffs = np.asarray(coeffs, np.float32)
    z0 = coeffs[:, 0] @ np.asarray(W_init, np.float32) + np.asarray(
        b_init, np.float32)                              # [B, H]
    dX = coeffs[:, 1:] - coeffs[:, :-1]                  # [B, T-1, C]

    shared = _shared_maps(W1, b1, W2, b2)
    in_maps = [dict(shared, **_prep_core(z0, dX, r)) for r in range(NCORES)]

    nc = _get_nc()
    res = run_bass_kernel_spmd(nc, in_maps, core_ids=list(range(NCORES)))

    z_T = np.empty((B, H), np.float32)
    for r in range(NCORES):
        o = res.results[r]["zT_out"]                     # [q, H, BCH]
        z_T[BLOC * r:BLOC * (r + 1)] = (
            o.transpose(0, 2, 1).reshape(BLOC, H))
    logits = z_T @ np.asarray(W_out, np.float32) + np.asarray(
        b_out, np.float32)
    m = logits.max(axis=-1, keepdims=True)
    e = np.exp(logits - m)
    out = e / e.sum(axis=-1, keepdims=True)
    if _want_results:
        return out.astype(np.float32), res
    return out.astype(np.float32)


# revision 48
# speedup vs baseline: 2.7212x; 2.7212x over previous
"""Neural CDE Trainium2 kernel.

Data-parallel over batch: 8 cores x 64 batch. Per core, two independent
"chains" of 32 batch elements run the 255-step Euler scan concurrently so
engine work from one chain hides the serial-dependency latency of the other.

Critical-cycle-oriented design (wall time ~= per-chain dependency cycle,
not engine throughput; fp32 everywhere - the CDE dynamics are chaotic
(error amplification ~1e8 over 255 steps) so bf16/fp32r anywhere explodes
the output):
  state s[f, b] = W1^T z + b1, updated incrementally via the linearity of
  MM1:  s_{k+1} = s_k + W1^T zinc_k.  The z state itself is accumulated
  OFF the critical path on Pool; only s is on the cycle.

Per-chain layouts (b in [0,32), h = 32*g + h_lo, g in [0,4)):
  ds = W1^T zit [128, 32]  PSUM, one K=128 matmul (stationary W1 [h, f])
  s  = s + ds   [128, 32]  SBUF (DVE; k=0 adds b1 per-partition instead)
  h = max(s, 0) [128, 32]  SBUF (DVE tensor_scalar)
  y = b2 + h@W2 [128, 384] PSUM, partitions p = 32*g + b, free = (h_lo, c);
                one K=4 bias-seed matmul (rank-4 factorization b2big =
                E4^T B4, start=True, no data deps so it runs early) + per
                group g a K=128 main matmul (stop=True); the 4 groups
                stream CONCURRENTLY via col tiling at tile_position
                (0, 32g) (HW-verified ~4x vs serial)
  t = tanh(y)   [128, 384] SBUF (ACT reads PSUM)
  u = t * dx    [128, 384] SBUF (chain 0: DVE, chain 1: Pool — splits the
                two chains' long tails across engine queues; dx broadcast
                over h_lo via 0-stride AP)
  zi            [128, 32]  SBUF (DVE reduce over innermost c) = [(g,b), h_lo]
  zit           [128, 32]  SBUF (DVE 32x32 block transpose) = [h, b] = zinc
  z_new = z + zit          (Pool, off critical path; z only needed at end)

z0, dX and the final readout/softmax are tiny (<0.01% of FLOPs) and run on
host numpy as part of sharding/gather.
"""

import numpy as np
from contextlib import ExitStack

B, T, C = 512, 256, 12
H, FF, O = 128, 128, 20
NCORES = 8
BLOC = B // NCORES          # 64 batch per core
NCHAIN = 2
BCH = BLOC // NCHAIN        # 32 batch per chain
NSTEP = T - 1               # 255
NG = 4                      # h groups of 32
FD = BCH * C                # 384 free elems for y/t/u tiles

_CACHE = {}


def _build(nsteps=NSTEP):
    import concourse.bass as bass
    import concourse.mybir as mybir
    import concourse.tile as tile
    import concourse.bacc as bacc

    f32 = mybir.dt.float32

    nc = bacc.Bacc("TRN2", target_bir_lowering=False, debug=False,
                   num_devices=NCORES)

    z0T_d = nc.dram_tensor("z0T", [NCHAIN, H, BCH], f32, kind="ExternalInput")
    W1_d = nc.dram_tensor("W1", [H, FF], f32, kind="ExternalInput")
    b1_d = nc.dram_tensor("b1", [FF, 1], f32, kind="ExternalInput")
    W2_d = nc.dram_tensor("W2", [FF, H * C], f32, kind="ExternalInput")
    # rank-4 factorization of the per-partition-group bias: y init = E4^T B4
    e4_d = nc.dram_tensor("e4", [NG, 128], f32, kind="ExternalInput")
    b4_d = nc.dram_tensor("b4", [NG, FD], f32, kind="ExternalInput")
    nd = NSTEP
    dx_d = nc.dram_tensor("dxs", [NCHAIN, 128, nd * C], f32,
                          kind="ExternalInput")
    zout_d = nc.dram_tensor("zT_out", [NCHAIN, H, BCH], f32,
                            kind="ExternalOutput")

    with tile.TileContext(nc) as tc, ExitStack() as ctx:
        consts = ctx.enter_context(tc.tile_pool(name="consts", bufs=1))

        w1 = consts.tile([H, FF], f32, tag="w1")
        nc.sync.dma_start(w1[:], W1_d[:])
        b1c = consts.tile([FF, 1], f32, tag="b1c")
        nc.sync.dma_start(b1c[:], b1_d[:])
        w2 = consts.tile([FF, H * C], f32, tag="w2")
        nc.sync.dma_start(w2[:], W2_d[:])
        e4 = consts.tile([NG, 128], f32, tag="e4")
        nc.sync.dma_start(e4[:], e4_d[:])
        b4 = consts.tile([NG, FD], f32, tag="b4")
        nc.sync.dma_start(b4[:], b4_d[:])
        dxs = [consts.tile([128, nd * C], f32, tag=f"dx{q}", name=f"dx{q}")
               for q in range(NCHAIN)]
        for q in range(NCHAIN):
            nc.sync.dma_start(dxs[q][:], dx_d[q])

        pools = {}
        for q in range(NCHAIN):
            pools[q] = {
                "zt": ctx.enter_context(tc.tile_pool(name=f"zt{q}", bufs=2)),
                "h": ctx.enter_context(tc.tile_pool(name=f"h{q}", bufs=2)),
                "t": ctx.enter_context(tc.tile_pool(name=f"t{q}", bufs=2)),
                "u": ctx.enter_context(tc.tile_pool(name=f"u{q}", bufs=2)),
                "zi": ctx.enter_context(tc.tile_pool(name=f"zi{q}", bufs=2)),
                "zit": ctx.enter_context(tc.tile_pool(name=f"zit{q}", bufs=2)),
                "s": ctx.enter_context(tc.tile_pool(name=f"ssb{q}", bufs=2)),
                "ds": ctx.enter_context(tc.tile_pool(
                    name=f"dsps{q}", bufs=2, space=bass.MemorySpace.PSUM)),
                "y": ctx.enter_context(tc.tile_pool(
                    name=f"yps{q}", bufs=2, space=bass.MemorySpace.PSUM)),
            }

        # initial z state; s = W1^T z + b1 lives in SBUF, updated via the
        # MM1 linearity trick each step
        zt, s_st = [], [None] * NCHAIN
        for q in range(NCHAIN):
            z = pools[q]["zt"].tile([H, BCH], f32, tag="zt", name=f"zt_init{q}")
            nc.sync.dma_start(z[:], z0T_d[q])
            zt.append(z)

        zit_prev = [None] * NCHAIN
        u_st = [None] * NCHAIN

        def seed(q):
            """y-tile bias seed via one K=4 matmul. Emitted for BOTH chains
            before either chain's stall-prone MM1-delta so the in-order PE
            fills its zit-wait window with this work."""
            p = pools[q]
            y = p["y"].tile([128, FD], f32, tag="y", name=f"y{q}")
            nc.tensor.matmul(y[:], e4[:], b4[:],
                             start=True, stop=False, skip_group_check=True)
            return y

        def head(q, k, y):
            """MM1 delta + s update + relu + MM2 (PE/DVE)."""
            p = pools[q]
            src = zt[q] if k == 0 else zit_prev[q]
            ds = p["ds"].tile([FF, BCH], f32, tag="ds")
            nc.tensor.matmul(ds[:], w1[:], src[:], start=True, stop=True)
            snew = p["s"].tile([FF, BCH], f32, tag="s")
            if k == 0:
                nc.vector.tensor_scalar(snew[:], ds[:], b1c[:], None,
                                        mybir.AluOpType.add)
            else:
                nc.vector.tensor_add(snew[:], s_st[q][:], ds[:])
            s_st[q] = snew
            h = p["h"].tile([FF, BCH], f32, tag="h")
            nc.vector.tensor_scalar(h[:], snew[:], 0.0, None,
                                    mybir.AluOpType.max)
            for g in range(NG):
                nc.tensor.matmul(y[32 * g:32 * (g + 1), :], h[:],
                                 w2[:, FD * g:FD * (g + 1)],
                                 start=False, stop=True,
                                 tile_position=(0, 32 * g),
                                 skip_group_check=True)
            return y

        def tanh_mul(q, k, y):
            """tanh (ACT) + dx multiply (chain 0: DVE, chain 1: Pool)."""
            p = pools[q]
            t = p["t"].tile([128, FD], f32, tag="t")
            nc.scalar.activation(t[:], y[:],
                                 mybir.ActivationFunctionType.Tanh)
            veng = nc.vector if q == 0 else nc.gpsimd
            u = p["u"].tile([128, FD], f32, tag="u")
            kk = k % nd
            dxk = dxs[q][:, C * kk:C * (kk + 1)]
            veng.tensor_tensor(
                u[:].rearrange("p (hl c) -> p hl c", c=C),
                t[:].rearrange("p (hl c) -> p hl c", c=C),
                dxk.unsqueeze(1).broadcast_to([128, BCH, C]),
                mybir.AluOpType.mult)
            u_st[q] = u

        def red_xp(q):
            """c-reduce + 32x32 block transpose (DVE) + z accumulate (Pool)."""
            p = pools[q]
            zi = p["zi"].tile([128, BCH], f32, tag="zi")
            nc.vector.reduce_sum(zi[:],
                                 u_st[q][:].rearrange("p (hl c) -> p hl c", c=C),
                                 axis=mybir.AxisListType.X)
            zit = p["zit"].tile([128, BCH], f32, tag="zit")
            nc.vector.transpose(zit[:], zi[:])
            zit_prev[q] = zit
            znew = p["zt"].tile([H, BCH], f32, tag="zt")
            nc.gpsimd.tensor_add(znew[:], zt[q][:], zit[:])
            zt[q] = znew

        for k in range(nsteps):
            # Per-chain-sequential emission. Measured better than every
            # alternative (heads-first grouping, full merge, cross-iteration
            # deferral): the chains self-organize into a half-cycle stagger
            # that aligned emission orders destroy.
            ys = [seed(q) for q in range(NCHAIN)]
            for q in range(NCHAIN):
                head(q, k, ys[q])
                tanh_mul(q, k, ys[q])
                red_xp(q)

        for q in range(NCHAIN):
            nc.sync.dma_start(zout_d[q], zt[q][:])

    nc.compile()
    return nc


def _get_nc(nsteps=NSTEP):
    key = ("nc", nsteps)
    if key not in _CACHE:
        _CACHE[key] = _build(nsteps)
    return _CACHE[key]


def _prep_core(z0, dX, r):
    """Per-core input map. z0 [B, H] fp32, dX [B, T-1, C] fp32."""
    z0c = z0[BLOC * r:BLOC * (r + 1)]          # [64, 128]
    # [q, b, h] -> [q, h, b]
    z0T = (z0c.reshape(NCHAIN, BCH, H)
              .transpose(0, 2, 1)
              .astype(np.float32, copy=True))
    dxc = dX[BLOC * r:BLOC * (r + 1)]          # [64, 255, 12]
    nd = NSTEP
    dxq = np.empty((NCHAIN, 128, nd * C), np.float32)
    for q in range(NCHAIN):
        blk = dxc[BCH * q:BCH * (q + 1), :nd]      # [32, nd, 12]
        dxq[q] = np.tile(blk.reshape(BCH, nd * C), (NG, 1))
    return {"z0T": np.ascontiguousarray(z0T), "dxs": np.ascontiguousarray(dxq)}


def _shared_maps(W1, b1, W2, b2):
    """Per-core replicated weight tensors."""
    b2m = np.asarray(b2, np.float32).reshape(NG, BCH * C)    # chunk g = rows
    e4 = np.kron(np.eye(NG, dtype=np.float32), np.ones((1, 32), np.float32))
    return {
        "W1": np.ascontiguousarray(np.asarray(W1, np.float32)),
        "b1": np.ascontiguousarray(np.asarray(b1, np.float32).reshape(FF, 1)),
        "W2": np.ascontiguousarray(np.asarray(W2, np.float32)),
        "e4": np.ascontiguousarray(e4.reshape(NG, 128)),
        "b4": np.ascontiguousarray(b2m),
    }


def kernel(coeffs, times, W_init, b_init, W1, b1, W2, b2, W_out, b_out,
           _want_results=False):
    from concourse.bass_utils import run_bass_kernel_spmd

    coeffs = np.asarray(coeffs, np.float32)
    z0 = coeffs[:, 0] @ np.asarray(W_init, np.float32) + np.asarray(
        b_init, np.float32)                              # [B, H]
    dX = coeffs[:, 1:] - coeffs[:, :-1]                  # [B, T-1, C]

    shared = _shared_maps(W1, b1, W2, b2)
    in_maps = [dict(shared, **_prep_core(z0, dX, r)) for r in range(NCORES)]

    nc = _get_nc()
    res = run_bass_kernel_spmd(nc, in_maps, core_ids=list(range(NCORES)))

    z_T = np.empty((B, H), np.float32)
    for r in range(NCORES):
        o = res.results[r]["zT_out"]                     # [q, H, BCH]
        z_T[BLOC * r:BLOC * (r + 1)] = (
            o.transpose(0, 2, 1).reshape(BLOC, H))
    logits = z_T @ np.asarray(W_out, np.float32) + np.asarray(
        b_out, np.float32)
    m = logits.max(axis=-1, keepdims=True)
    e = np.exp(logits - m)
    out = e / e.sum(axis=-1, keepdims=True)
    if _want_results:
        return out.astype(np.float32), res
    return out.astype(np.float32)
